# revision 18
# baseline (speedup 1.0000x reference)
"""Trainium2 Bass kernel for a 2-layer LSTM LM with full-vocab softmax.

Model: V=32000, E=256, H=512, L=2, B=16, S=128.
  xs = emb[y_target]                      (host-side gather)
  2-layer LSTM over S steps               (replicated on all 8 cores)
  probs = softmax(h1 @ Wout.T + bout)     (vocab-sharded: 4000 vocab rows/core)

Per-core device program (SPMD, identical; per-core Wout slice arrives as input):
  A : xg0 = Wih0T.T @ xsT  (+b0)  for all 2048 tokens   (batched, efficient)
  B : the two layer recurrences run INTERLEAVED (layer 1 trails layer 0 by
      LAG steps; the input-side gates xg1 for layer 1 are produced in chunks
      as layer 0's h stream becomes available), so the two independent
      dependency chains fill each other's ACT/DVE gaps.
  E : logits slice -> exp (partial denominators via accum_out)
      -> ONE AllReduce of softmax denominators per token-half -> scale -> out

Cell trick: only sigmoid is used on the ACT engine.  Host pre-scales the
g-gate rows of the weights by 2 and the initial c by 2 (C := 2c), so
  tanh(g)   = 2*sig(2g) - 1      (2g comes out of the matmul directly)
  C_new     = sig_f*C + sig_i*(4*sig(2g) - 2)
  tanh(c)   = 2*sig(C_new) - 1
which needs exactly two ACT ops per step: sig over all 256 gate cols (read
straight from PSUM: xg_t is preloaded into PSUM by an identity matmul) and
sig over C_new.

Token index t = s*B + b.  Gate tile order (128-row tiles): [i0..i3 f0..f3
o0..o3 g0..g3] so one sigmoid covers contiguous columns.
"""

import numpy as np
import ml_dtypes

import concourse.bass as bass
import concourse.mybir as mybir
import concourse.tile as tile
from concourse import bacc
from concourse.bass_utils import run_bass_kernel_spmd

V, E, H = 32000, 256, 512
B, S = 16, 128
T = S * B              # 2048 tokens
G = 4 * H              # 2048 gates
P = 128
NCORES = 8
VL = V // NCORES       # 4000 vocab rows per core
NT_E = 4               # vocab chunks per core in phase E
VC = VL // NT_E        # 1000 vocab cols per chunk
MT_E = T // P          # 16 token tiles of 128
HALF_MT = MT_E // 2    # 8 token tiles per half
LAG = 34               # layer-1 recurrence trails layer-0 by this many steps

bf16 = mybir.dt.bfloat16
f16 = mybir.dt.float16
f32 = mybir.dt.float32
AF = mybir.ActivationFunctionType
ALU = mybir.AluOpType
AX = mybir.AxisListType

_nbf16 = ml_dtypes.bfloat16


def _gate_perm():
    """Row permutation of the [4H] gate dim: [i f o g].

    PyTorch gate order: i[0:512) f[512:1024) g[1024:1536) o[1536:2048).
    """
    idx = []
    for base in (0, 512, 1536, 1024):   # i, f, o, g
        idx.extend(range(base, base + 512))
    return np.array(idx, dtype=np.int64)


_PERM = _gate_perm()


class _Rec:
    """State of one layer's recurrence (emitted one step at a time)."""

    def __init__(self, nc, whhT, xg, h_all, c_init_dram, ident, cell_pool,
                 ps_pool, tag):
        self.nc = nc
        self.whhT = whhT
        self.xg = xg
        self.h_all = h_all
        self.ident = ident
        self.cell = cell_pool
        self.ps = ps_pool
        self.tag = tag
        self.c_prev = cell_pool.tile([P, 4, B], f32, tag=f"c{tag}")
        nc.sync.dma_start(self.c_prev[:],
                          c_init_dram.rearrange("(k p) b -> p k b", p=P))

    def step(self, t):
        nc = self.nc
        pst = self.ps.tile([P, 256], f32, tag=f"g{self.tag}")
        tsl = slice(t * B, (t + 1) * B)
        # xg lives in a 2-chunk (64-step) ring
        tr = t % 64
        xsl = slice(tr * B, (tr + 1) * B)
        # preload xg_t into PSUM via identity matmul, then accumulate Whh MMs
        for mt in range(16):
            csl = slice(mt * B, (mt + 1) * B)
            nc.tensor.matmul(pst[:, csl], lhsT=self.ident[:],
                             rhs=self.xg[:, mt, xsl], start=True, stop=False)
            for kt in range(4):
                nc.tensor.matmul(
                    pst[:, csl],
                    lhsT=self.whhT[:, kt, mt * P:(mt + 1) * P],
                    rhs=self.h_all[:, kt, tsl],
                    start=False, stop=(kt == 3))
        # sig over all gates [i f o g] straight from PSUM
        sig = self.cell.tile([P, 256], f32, tag=f"sig{self.tag}")
        nc.scalar.activation(sig[:], pst[:], AF.Sigmoid)
        sig3 = sig.rearrange("p (k b) -> p k b", b=B)
        # G = 4*sig(2g) - 2  (= 2*tanh(g))
        Gt = self.cell.tile([P, 4, B], f32, tag=f"G{self.tag}")
        nc.vector.tensor_scalar(Gt[:], sig3[:, 12:16], 4.0, -2.0,
                                ALU.mult, ALU.add)
        # m2 = sig_i * G ; t1 = sig_f * C   (on gpsimd - frees DVE)
        m2 = self.cell.tile([P, 4, B], f32, tag=f"m2{self.tag}")
        nc.gpsimd.tensor_tensor(m2[:], sig3[:, 0:4], Gt[:], ALU.mult)
        t1 = self.cell.tile([P, 4, B], f32, tag=f"t1{self.tag}")
        nc.gpsimd.tensor_tensor(t1[:], sig3[:, 4:8], self.c_prev[:], ALU.mult)
        cn = self.cell.tile([P, 4, B], f32, tag=f"c{self.tag}")
        nc.vector.tensor_tensor(cn[:], t1[:], m2[:], ALU.add)
        self.c_prev = cn
        # tanh(c) = Tanh(C_new * 0.5)  (tanh co-resides in the sigmoid table
        # set, so no table switch); h = sig_o * tanh(c)
        hp = self.cell.tile([P, 4, B], f32, tag=f"hp{self.tag}")
        nc.scalar.activation(hp[:], cn[:], AF.Tanh, scale=0.5)
        nc.vector.tensor_tensor(self.h_all[:, :, (t + 1) * B:(t + 2) * B],
                                sig3[:, 8:12], hp[:], ALU.mult)


def _gates_chunk(nc, wT, rhs_sb, n_kt, xg, bias_sb, ps_pool, ntk, use_act):
    """xg[:, mt, ring slot] = wT.T @ rhs + bias for one 512-token chunk.

    xg is a 2-chunk ring [P, 16, 1024]; chunk ntk goes to slot ntk % 2.
    """
    csl = slice(ntk * 512, (ntk + 1) * 512)       # source tokens
    osl = slice((ntk % 2) * 512, (ntk % 2) * 512 + 512)
    for mt in range(16):
        pst = ps_pool.tile([P, 2, 512], f32, tag="eps", name="gps")[:, 0, :]
        for kt in range(n_kt):
            nc.tensor.matmul(
                pst[:], lhsT=wT[:, kt, mt * P:(mt + 1) * P],
                rhs=rhs_sb[:, kt, csl],
                start=(kt == 0), stop=(kt == n_kt - 1))
        if (mt + use_act) % 2 == 0:
            nc.scalar.activation(xg[:, mt, osl], pst[:], AF.Identity,
                                 bias=bias_sb[:, mt:mt + 1])
        else:
            nc.vector.tensor_scalar_add(xg[:, mt, osl], pst[:],
                                        bias_sb[:, mt:mt + 1])


def build_kernel(bout_nonzero, timing_mode=False, stop_after=99):
    nc = bacc.Bacc("TRN2", target_bir_lowering=False, debug=False,
                   num_devices=1 if timing_mode else NCORES)

    # ---- DRAM I/O ----
    d_xsT = nc.dram_tensor("xsT", [E, T], bf16, kind="ExternalInput")
    d_wih0T = nc.dram_tensor("wih0T", [E, G], bf16, kind="ExternalInput")
    d_whh0T = nc.dram_tensor("whh0T", [H, G], bf16, kind="ExternalInput")
    d_wih1T = nc.dram_tensor("wih1T", [H, G], bf16, kind="ExternalInput")
    d_whh1T = nc.dram_tensor("whh1T", [H, G], bf16, kind="ExternalInput")
    d_b0 = nc.dram_tensor("b0", [G], f32, kind="ExternalInput")
    d_b1 = nc.dram_tensor("b1", [G], f32, kind="ExternalInput")
    d_h0 = nc.dram_tensor("h0b", [H, B], bf16, kind="ExternalInput")
    d_c0 = nc.dram_tensor("c0f", [H, B], f32, kind="ExternalInput")
    d_h1 = nc.dram_tensor("h1b", [H, B], bf16, kind="ExternalInput")
    d_c1 = nc.dram_tensor("c1f", [H, B], f32, kind="ExternalInput")
    d_id = nc.dram_tensor("ident", [P, P], bf16, kind="ExternalInput")
    d_woutT = nc.dram_tensor("woutT", [H, VL], bf16, kind="ExternalInput")
    d_bout = nc.dram_tensor("boutv", [1, VL], bf16, kind="ExternalInput")
    d_out = nc.dram_tensor("out", [T, VL], f32, kind="ExternalOutput")

    with tile.TileContext(nc) as tc:
        with (
            tc.tile_pool(name="persist", bufs=1) as persist,
            tc.tile_pool(name="psum", bufs=2, space="PSUM") as psp,
            tc.tile_pool(name="dram", bufs=1, space="DRAM") as dram_pool,
        ):
            h1_all = persist.tile([P, 4, B * (S + 1)], bf16)
            nc.sync.dma_start(h1_all[:, :, 0:B],
                              d_h1.rearrange("(k p) b -> p k b", p=P))

            with (
                tc.tile_pool(name="wts", bufs=1) as wts,
                tc.tile_pool(name="cell", bufs=3) as cell_pool,
            ):
                # load weights / inputs
                xsT = wts.tile([P, 2, T], bf16)
                nc.sync.dma_start(xsT[:], d_xsT.rearrange("(k p) m -> p k m", p=P))
                wih0T = wts.tile([P, 2, G], bf16)
                nc.sync.dma_start(wih0T[:], d_wih0T.rearrange("(k p) m -> p k m", p=P))
                whh0T = wts.tile([P, 4, G], bf16)
                nc.sync.dma_start(whh0T[:], d_whh0T.rearrange("(k p) m -> p k m", p=P))
                wih1T = wts.tile([P, 4, G], bf16)
                nc.sync.dma_start(wih1T[:], d_wih1T.rearrange("(k p) m -> p k m", p=P))
                whh1T = wts.tile([P, 4, G], bf16)
                nc.sync.dma_start(whh1T[:], d_whh1T.rearrange("(k p) m -> p k m", p=P))
                b0sb = wts.tile([P, 16], f32)
                nc.sync.dma_start(b0sb[:], d_b0.rearrange("(m p) -> p m", p=P))
                b1sb = wts.tile([P, 16], f32)
                nc.sync.dma_start(b1sb[:], d_b1.rearrange("(m p) -> p m", p=P))
                ident = wts.tile([P, P], bf16)
                nc.sync.dma_start(ident[:], d_id[:])

                xg0 = wts.tile([P, 16, 1024], bf16, tag="xg0")
                xg1 = wts.tile([P, 16, 1024], f16, tag="xg1")
                h0_all = wts.tile([P, 4, B * (S + 1)], bf16)
                nc.sync.dma_start(h0_all[:, :, 0:B],
                                  d_h0.rearrange("(k p) b -> p k b", p=P))

                # Phase A: first two xg0 chunks up front, rest in the loop
                _gates_chunk(nc, wih0T, xsT, 2, xg0, b0sb, psp, 0, 0)
                _gates_chunk(nc, wih0T, xsT, 2, xg0, b0sb, psp, 1, 0)

                rec0 = _Rec(nc, whh0T, xg0, h0_all, d_c0, ident, cell_pool,
                            psp, 0)
                rec1 = _Rec(nc, whh1T, xg1, h1_all, d_c1, ident, cell_pool,
                            psp, 1)
                h0_tok = h0_all[:, :, B:B * (S + 1)]

                do_l0 = stop_after >= 2
                do_C = stop_after >= 3
                do_l1 = stop_after >= 4
                for tt in range(S + LAG):
                    if tt < S and do_l0:
                        rec0.step(tt)
                    if tt % 32 == 0 and 0 < tt:
                        if do_l0 and tt // 32 + 1 <= 3:
                            # refill xg0 ring (chunk tt//32+1)
                            _gates_chunk(nc, wih0T, xsT, 2, xg0, b0sb, psp,
                                         tt // 32 + 1, 0)
                        if do_C and tt <= 128:
                            _gates_chunk(nc, wih1T, h0_tok, 4, xg1, b1sb, psp,
                                         tt // 32 - 1, 1)
                    if do_l1 and tt >= LAG:
                        rec1.step(tt - LAG)

            # ---- Phase E: output projection + softmax (vocab-sharded) ----
            if stop_after < 5:
                nc.gpsimd.dma_start(d_out[0:P, 0:4], h1_all[:, 0, 0:4])
            else:
              with tc.tile_pool(name="ephase", bufs=1) as ep, \
                   tc.tile_pool(name="ework", bufs=2) as ew:
                h1_tok = h1_all[:, :, B:B * (S + 1)]
                bout_sb = None
                if bout_nonzero:
                    bout_sb = ep.tile([1, VL], bf16)
                    nc.sync.dma_start(bout_sb[:], d_bout[:])
                    ones_sb = ep.tile([1, P], bf16)
                    nc.vector.memset(ones_sb[:], 1.0)

                for half in range(2):
                    etile = ep.tile([P, HALF_MT, VL], f16, tag="exp")
                    dn = ep.tile([P, HALF_MT, NT_E], f32, tag="dn")
                    for ntk in range(NT_E):
                        wch = ew.tile([P, 4, VC], bf16, tag="wout")
                        nc.sync.dma_start(
                            wch[:],
                            d_woutT.rearrange("(k p) v -> p k v", p=P)[
                                :, :, ntk * VC:(ntk + 1) * VC])
                        for mt in range(HALF_MT):
                            tok0 = (half * HALF_MT + mt) * P
                            pst = psp.tile([P, 2, 512], f32, tag="eps")
                            for sub in range(2):
                                for kt in range(4):
                                    nc.tensor.matmul(
                                        pst[:, sub, 0:500],
                                        lhsT=h1_tok[:, kt, tok0:tok0 + P],
                                        rhs=wch[:, kt, sub * 500:(sub + 1) * 500],
                                        start=(kt == 0),
                                        stop=(kt == 3 and not bout_nonzero))
                                if bout_nonzero:
                                    nc.tensor.matmul(
                                        pst[:, sub, 0:500], lhsT=ones_sb[:],
                                        rhs=bout_sb[:, ntk * VC + sub * 500:
                                                    ntk * VC + (sub + 1) * 500],
                                        start=False, stop=True)
                            nc.scalar.activation(
                                etile[:, mt, ntk * VC:(ntk + 1) * VC]
                                .rearrange("p (s v) -> p s v", v=500),
                                pst[:, :, 0:500], AF.Exp,
                                accum_out=dn[:, mt, ntk:ntk + 1])
                    # global softmax denominators: one AllReduce per half
                    dnh = ep.tile([P, HALF_MT], f32, tag="dnh")
                    nc.vector.tensor_reduce(dnh[:], dn[:], AX.X, ALU.add)
                    if timing_mode:
                        dng = dnh
                    else:
                        cc_in = dram_pool.tile([P, HALF_MT], f32, tag="ccin")
                        cc_out = dram_pool.tile([P, HALF_MT], f32, tag="ccout")
                        nc.sync.dma_start(cc_in[:], dnh[:])
                        nc.gpsimd.collective_compute(
                            "AllReduce", ALU.add,
                            replica_groups=[list(range(NCORES))],
                            ins=[cc_in.opt()], outs=[cc_out.opt()])
                        dng = ep.tile([P, HALF_MT], f32, tag="dng")
                        nc.sync.dma_start(dng[:], cc_out[:])
                    rec = ep.tile([P, HALF_MT], f32, tag="rec")
                    nc.vector.reciprocal(rec[:], dng[:])
                    for mt in range(HALF_MT):
                        tok0 = (half * HALF_MT + mt) * P
                        stage = ew.tile([P, VL], f32, tag="stage")
                        nc.vector.tensor_scalar_mul(stage[:], etile[:, mt, :],
                                                    rec[:, mt:mt + 1])
                        eng = nc.sync if mt % 2 == 0 else nc.gpsimd
                        eng.dma_start(d_out[tok0:tok0 + P, :], stage[:])
    nc.finalize()
    return nc


_CACHE = {}


def kernel(y_target, emb, Wih0, Whh0, bih0, bhh0, Wih1, Whh1, bih1, bhh1,
           Wout, bout, h0, c0):
    y = np.asarray(y_target)
    emb = np.asarray(emb, dtype=np.float32)
    xs = emb[y]                                   # [B, S, E]
    xsT = np.ascontiguousarray(
        np.transpose(xs, (2, 1, 0)).reshape(E, T))  # [E, T], t = s*B+b

    # g-gate rows (last 512 after permutation) x2 so tanh(g) = 2*sig(2g)-1
    gs = np.ones((G, 1), np.float32)
    gs[1536:] = 2.0
    b0 = ((np.asarray(bih0) + np.asarray(bhh0)).astype(np.float32)[_PERM]
          * gs[:, 0])
    b1 = ((np.asarray(bih1) + np.asarray(bhh1)).astype(np.float32)[_PERM]
          * gs[:, 0])
    wih0T = np.ascontiguousarray(
        (np.asarray(Wih0, np.float32)[_PERM] * gs).T).astype(_nbf16)
    whh0T = np.ascontiguousarray(
        (np.asarray(Whh0, np.float32)[_PERM] * gs).T).astype(_nbf16)
    wih1T = np.ascontiguousarray(
        (np.asarray(Wih1, np.float32)[_PERM] * gs).T).astype(_nbf16)
    whh1T = np.ascontiguousarray(
        (np.asarray(Whh1, np.float32)[_PERM] * gs).T).astype(_nbf16)

    h0 = np.asarray(h0, dtype=np.float32)
    c0 = np.asarray(c0, dtype=np.float32)
    bout = np.asarray(bout, dtype=np.float32)
    Wout = np.asarray(Wout, dtype=np.float32)

    bout_nonzero = bool(np.any(bout != 0.0))
    key = bout_nonzero
    if key not in _CACHE:
        _CACHE[key] = build_kernel(bout_nonzero)
    nc = _CACHE[key]

    common = {
        "xsT": xsT.astype(_nbf16),
        "wih0T": wih0T, "whh0T": whh0T, "wih1T": wih1T, "whh1T": whh1T,
        "b0": b0, "b1": b1,
        "h0b": np.ascontiguousarray(h0[0].T).astype(_nbf16),
        "c0f": np.ascontiguousarray(2.0 * c0[0].T).astype(np.float32),
        "h1b": np.ascontiguousarray(h0[1].T).astype(_nbf16),
        "c1f": np.ascontiguousarray(2.0 * c0[1].T).astype(np.float32),
        "ident": np.eye(P, dtype=_nbf16),
    }
    in_maps = []
    for k in range(NCORES):
        vs = slice(k * VL, (k + 1) * VL)
        m = dict(common)
        m["woutT"] = np.ascontiguousarray(Wout[vs].T).astype(_nbf16)
        m["boutv"] = bout[None, vs].astype(_nbf16)
        in_maps.append(m)

    import os
    trace = bool(os.environ.get("KERNEL_TRACE"))
    res = run_bass_kernel_spmd(nc, in_maps, core_ids=list(range(NCORES)),
                               trace=trace)
    global LAST_EXEC_NS
    LAST_EXEC_NS = res.exec_time_ns
    full = np.concatenate([r["out"] for r in res.results], axis=1)  # [T, V]
    return np.ascontiguousarray(
        full.reshape(S, B, V).transpose(1, 0, 2)).astype(np.float32)


if __name__ == "__main__":
    import reference
    inputs = {k: np.asarray(v) for k, v in reference.setup_inputs().items()}
    out = kernel(**inputs)
    print("kernel out", out.shape, out.dtype)


# revision 37
# speedup vs baseline: 1.1272x; 1.1272x over previous
"""Trainium2 Bass kernel for a 2-layer LSTM LM with full-vocab softmax.

Model: V=32000, E=256, H=512, L=2, B=16, S=128.
  xs = emb[y_target]                      (host-side gather)
  2-layer LSTM over S steps               (replicated on all 8 cores)
  probs = softmax(h1 @ Wout.T + bout)     (vocab-sharded: 4000 vocab rows/core)

Per-core device program (SPMD, identical; per-core Wout slice arrives as input):
  A : xg0 = Wih0T.T @ xsT  (+b0)  for all 2048 tokens   (batched, efficient)
  B : the two layer recurrences run INTERLEAVED (layer 1 trails layer 0 by
      LAG steps; the input-side gates xg1 for layer 1 are produced in chunks
      as layer 0's h stream becomes available), so the two independent
      dependency chains fill each other's ACT/DVE gaps.
  E : logits slice -> exp (partial denominators via accum_out)
      -> ONE AllReduce of softmax denominators per token-half -> scale -> out

Cell trick: only sigmoid is used on the ACT engine.  Host pre-scales the
g-gate rows of the weights by 2 and the initial c by 2 (C := 2c), so
  tanh(g)   = 2*sig(2g) - 1      (2g comes out of the matmul directly)
  C_new     = sig_f*C + sig_i*(4*sig(2g) - 2)
  tanh(c)   = 2*sig(C_new) - 1
which needs exactly two ACT ops per step: sig over all 256 gate cols (read
straight from PSUM: xg_t is preloaded into PSUM by an identity matmul) and
sig over C_new.

Token index t = s*B + b.  Gate tile order (128-row tiles): [i0..i3 f0..f3
o0..o3 g0..g3] so one sigmoid covers contiguous columns.
"""

import numpy as np
import ml_dtypes

import concourse.bass as bass
import concourse.mybir as mybir
import concourse.tile as tile
from concourse import bacc
from concourse.bass_utils import run_bass_kernel_spmd

V, E, H = 32000, 256, 512
B, S = 16, 128
T = S * B              # 2048 tokens
G = 4 * H              # 2048 gates
P = 128
NCORES = 8
VL = V // NCORES       # 4000 vocab rows per core
NT_E = 4               # vocab chunks per core in phase E
VC = VL // NT_E        # 1000 vocab cols per chunk
MT_E = T // P          # 16 token tiles of 128
HALF_MT = MT_E // 2    # 8 token tiles per half
LAG = 18               # layer-1 recurrence trails layer-0 by this many steps
CCH = 16               # xg1 production chunk, in steps (16 tokens each)

bf16 = mybir.dt.bfloat16
f16 = mybir.dt.float16
f32 = mybir.dt.float32
AF = mybir.ActivationFunctionType
ALU = mybir.AluOpType
AX = mybir.AxisListType

_nbf16 = ml_dtypes.bfloat16


def _gate_perm():
    """Row permutation of the [4H] gate dim: [i f o g].

    PyTorch gate order: i[0:512) f[512:1024) g[1024:1536) o[1536:2048).
    """
    idx = []
    for base in (0, 512, 1536, 1024):   # i, f, o, g
        idx.extend(range(base, base + 512))
    return np.array(idx, dtype=np.int64)


_PERM = _gate_perm()


class _Rec:
    """State of one layer's recurrence (emitted one step at a time)."""

    def __init__(self, nc, whhT, xg, h_all, c_init_dram, ident, cell_pool,
                 ps_pool, tag, ring_steps):
        self.nc = nc
        self.whhT = whhT
        self.xg = xg
        self.h_all = h_all
        self.ident = ident
        self.cell = cell_pool
        self.ps = ps_pool
        self.tag = tag
        self.ring_steps = ring_steps
        self.c_prev = cell_pool.tile([P, 4, B], f32, tag=f"c{tag}")
        nc.sync.dma_start(self.c_prev[:],
                          c_init_dram.rearrange("(k p) b -> p k b", p=P))

    def step(self, t):
        nc = self.nc
        pst = self.ps.tile([P, 256], f32, tag=f"g{self.tag}")
        tsl = slice(t * B, (t + 1) * B)
        tr = t % self.ring_steps
        xsl = slice(tr * B, (tr + 1) * B)
        # preload all of xg_t into PSUM via ONE identity matmul (3D moving
        # AP over the 16 mt tiles), then accumulate the Whh MMs per tile
        nc.tensor.matmul(pst.rearrange("p (m b) -> p m b", b=B),
                         lhsT=self.ident[:], rhs=self.xg[:, :, xsl],
                         start=True, stop=False)
        for mt in range(16):
            csl = slice(mt * B, (mt + 1) * B)
            for kt in range(4):
                nc.tensor.matmul(
                    pst[:, csl],
                    lhsT=self.whhT[:, kt, mt * P:(mt + 1) * P],
                    rhs=self.h_all[:, kt, tsl],
                    start=False, stop=(kt == 3), skip_group_check=True)
        # sig over all gates [i f o g] straight from PSUM
        sig = self.cell.tile([P, 256], f32, tag=f"sig{self.tag}")
        nc.scalar.activation(sig[:], pst[:], AF.Sigmoid)
        sig3 = sig.rearrange("p (k b) -> p k b", b=B)
        # G = 4*sig(2g) - 2  (= 2*tanh(g))
        Gt = self.cell.tile([P, 4, B], f32, tag=f"G{self.tag}")
        nc.vector.tensor_scalar(Gt[:], sig3[:, 12:16], 4.0, -2.0,
                                ALU.mult, ALU.add)
        # m2 = sig_i * G ; t1 = sig_f * C   (on gpsimd - frees DVE)
        m2 = self.cell.tile([P, 4, B], f32, tag=f"m2{self.tag}")
        nc.gpsimd.tensor_tensor(m2[:], sig3[:, 0:4], Gt[:], ALU.mult)
        t1 = self.cell.tile([P, 4, B], f32, tag=f"t1{self.tag}")
        nc.gpsimd.tensor_tensor(t1[:], sig3[:, 4:8], self.c_prev[:], ALU.mult)
        cn = self.cell.tile([P, 4, B], f32, tag=f"c{self.tag}")
        nc.vector.tensor_tensor(cn[:], t1[:], m2[:], ALU.add)
        self.c_prev = cn
        # tanh(c) = Tanh(C_new * 0.5)  (tanh co-resides in the sigmoid table
        # set, so no table switch); h = sig_o * tanh(c)
        hp = self.cell.tile([P, 4, B], f32, tag=f"hp{self.tag}")
        nc.scalar.activation(hp[:], cn[:], AF.Tanh, scale=0.5)
        nc.vector.tensor_tensor(self.h_all[:, :, (t + 1) * B:(t + 2) * B],
                                sig3[:, 8:12], hp[:], ALU.mult)


def _gates_chunk(nc, wT, rhs_sb, n_kt, xg, bias_sb, ps_pool, ntk, use_act,
                 csize=512):
    """xg[:, mt, ring slot] = wT.T @ rhs + bias for one csize-token chunk.

    xg is a 2-chunk ring [P, 16, 2*csize]; chunk ntk goes to slot ntk % 2.
    """
    csl = slice(ntk * csize, (ntk + 1) * csize)   # source tokens
    osl = slice((ntk % 2) * csize, (ntk % 2) * csize + csize)
    for mt in range(16):
        pst = ps_pool.tile([P, 2, 512], f32, tag="eps", name="gps")[:, 0, 0:csize]
        for kt in range(n_kt):
            nc.tensor.matmul(
                pst[:], lhsT=wT[:, kt, mt * P:(mt + 1) * P],
                rhs=rhs_sb[:, kt, csl],
                start=(kt == 0), stop=(kt == n_kt - 1))
        if (mt + use_act) % 2 == 0:
            nc.scalar.activation(xg[:, mt, osl], pst[:], AF.Identity,
                                 bias=bias_sb[:, mt:mt + 1])
        else:
            nc.vector.tensor_scalar_add(xg[:, mt, osl], pst[:],
                                        bias_sb[:, mt:mt + 1])


def build_kernel(bout_nonzero, timing_mode=False, stop_after=99):
    nc = bacc.Bacc("TRN2", target_bir_lowering=False, debug=False,
                   num_devices=1 if timing_mode else NCORES)

    # ---- DRAM I/O ----
    d_xsT = nc.dram_tensor("xsT", [E, T], bf16, kind="ExternalInput")
    d_wih0T = nc.dram_tensor("wih0T", [E, G], bf16, kind="ExternalInput")
    d_whh0T = nc.dram_tensor("whh0T", [H, G], bf16, kind="ExternalInput")
    d_wih1T = nc.dram_tensor("wih1T", [H, G], bf16, kind="ExternalInput")
    d_whh1T = nc.dram_tensor("whh1T", [H, G], bf16, kind="ExternalInput")
    d_b0 = nc.dram_tensor("b0", [G], f32, kind="ExternalInput")
    d_b1 = nc.dram_tensor("b1", [G], f32, kind="ExternalInput")
    d_h0 = nc.dram_tensor("h0b", [H, B], bf16, kind="ExternalInput")
    d_c0 = nc.dram_tensor("c0f", [H, B], f32, kind="ExternalInput")
    d_h1 = nc.dram_tensor("h1b", [H, B], bf16, kind="ExternalInput")
    d_c1 = nc.dram_tensor("c1f", [H, B], f32, kind="ExternalInput")
    d_id = nc.dram_tensor("ident", [P, P], bf16, kind="ExternalInput")
    d_woutT = nc.dram_tensor("woutT", [H, VL], bf16, kind="ExternalInput")
    d_bout = nc.dram_tensor("boutv", [1, VL], bf16, kind="ExternalInput")
    d_out = nc.dram_tensor("out", [T, VL], f32, kind="ExternalOutput")

    with tile.TileContext(nc) as tc:
        with (
            tc.tile_pool(name="persist", bufs=1) as persist,
            tc.tile_pool(name="psum", bufs=2, space="PSUM") as psp,
            tc.tile_pool(name="dram", bufs=1, space="DRAM") as dram_pool,
        ):
            h1_all = persist.tile([P, 4, B * (S + 1)], bf16)
            nc.sync.dma_start(h1_all[:, :, 0:B],
                              d_h1.rearrange("(k p) b -> p k b", p=P))

            with (
                tc.tile_pool(name="wts", bufs=1) as wts,
                tc.tile_pool(name="cell", bufs=3) as cell_pool,
            ):
                # load weights / inputs
                xsT = wts.tile([P, 2, T], bf16)
                nc.sync.dma_start(xsT[:], d_xsT.rearrange("(k p) m -> p k m", p=P))
                wih0T = wts.tile([P, 2, G], bf16)
                nc.sync.dma_start(wih0T[:], d_wih0T.rearrange("(k p) m -> p k m", p=P))
                whh0T = wts.tile([P, 4, G], bf16)
                nc.sync.dma_start(whh0T[:], d_whh0T.rearrange("(k p) m -> p k m", p=P))
                wih1T = wts.tile([P, 4, G], bf16)
                nc.sync.dma_start(wih1T[:], d_wih1T.rearrange("(k p) m -> p k m", p=P))
                whh1T = wts.tile([P, 4, G], bf16)
                nc.sync.dma_start(whh1T[:], d_whh1T.rearrange("(k p) m -> p k m", p=P))
                b0sb = wts.tile([P, 16], f32)
                nc.sync.dma_start(b0sb[:], d_b0.rearrange("(m p) -> p m", p=P))
                b1sb = wts.tile([P, 16], f32)
                nc.sync.dma_start(b1sb[:], d_b1.rearrange("(m p) -> p m", p=P))
                ident = wts.tile([P, P], bf16)
                nc.sync.dma_start(ident[:], d_id[:])

                xg0 = wts.tile([P, 16, 1024], bf16, tag="xg0")
                xg1 = wts.tile([P, 16, 2 * CCH * B], f16, tag="xg1")
                h0_all = wts.tile([P, 4, B * (S + 1)], bf16)
                nc.sync.dma_start(h0_all[:, :, 0:B],
                                  d_h0.rearrange("(k p) b -> p k b", p=P))

                # Phase A: first two xg0 chunks up front, rest in the loop
                _gates_chunk(nc, wih0T, xsT, 2, xg0, b0sb, psp, 0, 0)
                _gates_chunk(nc, wih0T, xsT, 2, xg0, b0sb, psp, 1, 0)

                rec0 = _Rec(nc, whh0T, xg0, h0_all, d_c0, ident, cell_pool,
                            psp, 0, 64)
                rec1 = _Rec(nc, whh1T, xg1, h1_all, d_c1, ident, cell_pool,
                            psp, 1, 2 * CCH)
                h0_tok = h0_all[:, :, B:B * (S + 1)]

                do_l0 = stop_after >= 2
                do_C = stop_after >= 3
                do_l1 = stop_after >= 4
                for tt in range(S + LAG):
                    if tt < S and do_l0:
                        rec0.step(tt)
                    if do_l0 and tt % 32 == 0 and 0 < tt and tt // 32 + 1 <= 3:
                        # refill xg0 ring (chunk tt//32+1)
                        _gates_chunk(nc, wih0T, xsT, 2, xg0, b0sb, psp,
                                     tt // 32 + 1, 0)
                    if do_C and tt % CCH == 0 and 0 < tt <= S:
                        _gates_chunk(nc, wih1T, h0_tok, 4, xg1, b1sb, psp,
                                     tt // CCH - 1, 1, csize=CCH * B)
                    if do_l1 and tt >= LAG:
                        rec1.step(tt - LAG)

            # ---- Phase E: output projection + softmax (vocab-sharded) ----
            if stop_after < 5:
                nc.gpsimd.dma_start(d_out[0:P, 0:4], h1_all[:, 0, 0:4])
            else:
              with tc.tile_pool(name="ephase", bufs=1) as ep, \
                   tc.tile_pool(name="ework", bufs=3) as ew:
                h1_tok = h1_all[:, :, B:B * (S + 1)]
                # resident Wout slice, loaded once in NT_E pieces
                wout_sb = ep.tile([P, 4, VL], bf16, tag="woutr")
                for ntk in range(NT_E):
                    nc.sync.dma_start(
                        wout_sb[:, :, ntk * VC:(ntk + 1) * VC],
                        d_woutT.rearrange("(k p) v -> p k v", p=P)[
                            :, :, ntk * VC:(ntk + 1) * VC])
                bout_sb = None
                if bout_nonzero:
                    bout_sb = ep.tile([1, VL], bf16)
                    nc.sync.dma_start(bout_sb[:], d_bout[:])
                    ones_sb = ep.tile([1, P], bf16)
                    nc.vector.memset(ones_sb[:], 1.0)

                QMT = 4   # token tiles per quarter
                for half in range(4):
                    etile = ep.tile([P, QMT, VL], f16, tag="exp")
                    dn = ep.tile([P, QMT, NT_E], f32, tag="dn")
                    for ntk in range(NT_E):
                        wch = wout_sb[:, :, ntk * VC:(ntk + 1) * VC]
                        for mt in range(QMT):
                            tok0 = (half * QMT + mt) * P
                            pst = psp.tile([P, 2, 512], f32, tag="eps")
                            for sub in range(2):
                                for kt in range(4):
                                    nc.tensor.matmul(
                                        pst[:, sub, 0:500],
                                        lhsT=h1_tok[:, kt, tok0:tok0 + P],
                                        rhs=wch[:, kt, sub * 500:(sub + 1) * 500],
                                        start=(kt == 0),
                                        stop=(kt == 3 and not bout_nonzero))
                                if bout_nonzero:
                                    nc.tensor.matmul(
                                        pst[:, sub, 0:500], lhsT=ones_sb[:],
                                        rhs=bout_sb[:, ntk * VC + sub * 500:
                                                    ntk * VC + (sub + 1) * 500],
                                        start=False, stop=True)
                            nc.scalar.activation(
                                etile[:, mt, ntk * VC:(ntk + 1) * VC]
                                .rearrange("p (s v) -> p s v", v=500),
                                pst[:, :, 0:500], AF.Exp,
                                accum_out=dn[:, mt, ntk:ntk + 1])
                    # global softmax denominators: one AllReduce per quarter
                    dnh = ep.tile([P, QMT], f32, tag="dnh")
                    nc.vector.tensor_reduce(dnh[:], dn[:], AX.X, ALU.add)
                    if timing_mode:
                        dng = dnh
                    else:
                        cc_in = dram_pool.tile([P, QMT], f32, tag=f"ccin{half}")
                        cc_out = dram_pool.tile([P, QMT], f32, tag=f"ccout{half}")
                        nc.sync.dma_start(cc_in[:], dnh[:])
                        nc.gpsimd.collective_compute(
                            "AllReduce", ALU.add,
                            replica_groups=[list(range(NCORES))],
                            ins=[cc_in.opt()], outs=[cc_out.opt()])
                        dng = ep.tile([P, QMT], f32, tag="dng")
                        nc.sync.dma_start(dng[:], cc_out[:])
                    rec = ep.tile([P, QMT], f32, tag="rec")
                    nc.vector.reciprocal(rec[:], dng[:])
                    for mt in range(QMT):
                        tok0 = (half * QMT + mt) * P
                        stage = ew.tile([P, VL], f32, tag="stage")
                        nc.vector.tensor_scalar_mul(stage[:], etile[:, mt, :],
                                                    rec[:, mt:mt + 1])
                        eng = nc.sync if mt % 2 == 0 else nc.gpsimd
                        eng.dma_start(d_out[tok0:tok0 + P, :], stage[:])
    nc.finalize()
    return nc


_CACHE = {}


def kernel(y_target, emb, Wih0, Whh0, bih0, bhh0, Wih1, Whh1, bih1, bhh1,
           Wout, bout, h0, c0):
    y = np.asarray(y_target)
    emb = np.asarray(emb, dtype=np.float32)
    xs = emb[y]                                   # [B, S, E]
    xsT = np.ascontiguousarray(
        np.transpose(xs, (2, 1, 0)).reshape(E, T))  # [E, T], t = s*B+b

    # g-gate rows (last 512 after permutation) x2 so tanh(g) = 2*sig(2g)-1
    gs = np.ones((G, 1), np.float32)
    gs[1536:] = 2.0
    b0 = ((np.asarray(bih0) + np.asarray(bhh0)).astype(np.float32)[_PERM]
          * gs[:, 0])
    b1 = ((np.asarray(bih1) + np.asarray(bhh1)).astype(np.float32)[_PERM]
          * gs[:, 0])
    wih0T = np.ascontiguousarray(
        (np.asarray(Wih0, np.float32)[_PERM] * gs).T).astype(_nbf16)
    whh0T = np.ascontiguousarray(
        (np.asarray(Whh0, np.float32)[_PERM] * gs).T).astype(_nbf16)
    wih1T = np.ascontiguousarray(
        (np.asarray(Wih1, np.float32)[_PERM] * gs).T).astype(_nbf16)
    whh1T = np.ascontiguousarray(
        (np.asarray(Whh1, np.float32)[_PERM] * gs).T).astype(_nbf16)

    h0 = np.asarray(h0, dtype=np.float32)
    c0 = np.asarray(c0, dtype=np.float32)
    bout = np.asarray(bout, dtype=np.float32)
    Wout = np.asarray(Wout, dtype=np.float32)

    bout_nonzero = bool(np.any(bout != 0.0))
    key = bout_nonzero
    if key not in _CACHE:
        _CACHE[key] = build_kernel(bout_nonzero)
    nc = _CACHE[key]

    common = {
        "xsT": xsT.astype(_nbf16),
        "wih0T": wih0T, "whh0T": whh0T, "wih1T": wih1T, "whh1T": whh1T,
        "b0": b0, "b1": b1,
        "h0b": np.ascontiguousarray(h0[0].T).astype(_nbf16),
        "c0f": np.ascontiguousarray(2.0 * c0[0].T).astype(np.float32),
        "h1b": np.ascontiguousarray(h0[1].T).astype(_nbf16),
        "c1f": np.ascontiguousarray(2.0 * c0[1].T).astype(np.float32),
        "ident": np.eye(P, dtype=_nbf16),
    }
    in_maps = []
    for k in range(NCORES):
        vs = slice(k * VL, (k + 1) * VL)
        m = dict(common)
        m["woutT"] = np.ascontiguousarray(Wout[vs].T).astype(_nbf16)
        m["boutv"] = bout[None, vs].astype(_nbf16)
        in_maps.append(m)

    import os
    trace = bool(os.environ.get("KERNEL_TRACE"))
    res = run_bass_kernel_spmd(nc, in_maps, core_ids=list(range(NCORES)),
                               trace=trace)
    global LAST_EXEC_NS
    LAST_EXEC_NS = res.exec_time_ns
    full = np.concatenate([r["out"] for r in res.results], axis=1)  # [T, V]
    return np.ascontiguousarray(
        full.reshape(S, B, V).transpose(1, 0, 2)).astype(np.float32)


if __name__ == "__main__":
    rng = np.random.default_rng(0)
    s = 0.02
    inputs = dict(
        y_target=rng.integers(0, V, (B, S)),
        emb=(rng.standard_normal((V, E)) * s).astype(np.float32),
        Wih0=(rng.standard_normal((G, E)) * s).astype(np.float32),
        Whh0=(rng.standard_normal((G, H)) * s).astype(np.float32),
        bih0=np.zeros(G, np.float32), bhh0=np.zeros(G, np.float32),
        Wih1=(rng.standard_normal((G, H)) * s).astype(np.float32),
        Whh1=(rng.standard_normal((G, H)) * s).astype(np.float32),
        bih1=np.zeros(G, np.float32), bhh1=np.zeros(G, np.float32),
        Wout=(rng.standard_normal((V, H)) * s).astype(np.float32),
        bout=np.zeros(V, np.float32),
        h0=(rng.standard_normal((2, B, H)) * s).astype(np.float32),
        c0=(rng.standard_normal((2, B, H)) * s).astype(np.float32),
    )
    out = kernel(**inputs)
    print("kernel out", out.shape, out.dtype)
